# revision 1
# baseline (speedup 1.0000x reference)
"""Trainium2 Bass kernel for nn_CausalFieldLayer.

Strategy (validated on host):
  * h = x@W_in is only consumed by three 1024->16 projections, so W_in folds
    into a single [1024,48] matrix W_all (plus a ones column for sum_d x).
  * The complex-octonion associator Jv is a fixed trilinear form; each cmul is
    computed as outer-product expansion (PE matmuls with 0/1 matrices),
    an elementwise multiply (DVE), and a contraction by G2 [256,16] (PE).
    The middle cmuls fuse contraction+re-expansion into single +-1 matrices
    (mrep = rep.G2^T, mtile = tile.G2^T) so U/Y never materialize. Jv is
    contracted twice: feature-major (G2 moving) and token-major (w as
    stationary, G2 chunks moving) so no PE transpose of Jv is needed.
  * Everything downstream of Jv is linear: J_expand/antisym/Pi_source/
    Pi_target/spinor-trace/W_out/alpha fold into P1,P2 [16,1024] on host.
  * The FFT conv is a 64-tap causal conv -> Toeplitz matmul on PE.
  * The layernorm mean is computed token-major on a side path (stationary
    JJc/pall slices x svec/sel17 vectors) and folded into the normalize
    scalars; variance uses the uncentered sum of squares. The residual x is
    added by an identity matmul accumulating into the output PSUM tile, and
    the normalize multiply doubles as the PSUM->SBUF evacuation.
  * Software pipelining: x DMA + fp32->bf16 convert (Pool) prefetch one
    chunk ahead; the transpose/projection front-end of chunk c+1 is emitted
    interleaved with chunk c's output phase so the in-order PE stream has
    independent work wherever the LN tail would stall it.
  * Data-parallel over B=8: core i handles batch element i.

Channel-major tensors use 32-aligned partition groups because matmul operand
base partitions must be in {0,32,64} and psum->sbuf copies cannot shift
partitions:
  pall psum/sbuf [80,Tc]: ps@0-15, sumx@16, pl@32-47, pa@64-79
  JJc [66,Tc] per chunk: Jv@0-15, Jc@32-47, ones@65 (gap rows zeroed)

The middle path runs in bf16 (validated: end-to-end error ~1e-3 relative,
because |out| ~ 0.07 * |x| so associator-path errors are damped 14x).
The residual enters as bf16 via the identity matmul; normalization stays
fp32 from PSUM.
"""

from contextlib import ExitStack

import numpy as np
import ml_dtypes

import concourse.bass as bass
import concourse.bacc as bacc
import concourse.mybir as mybir
import concourse.tile as tile
from concourse.bass_utils import run_bass_kernel_spmd

BF = ml_dtypes.bfloat16
F32 = np.float32

B, N, DM = 8, 4096, 1024
NCORES = 8
KSIZE = 64

EPS = 1e-5


# ----------------------------------------------------------------------------
# Host-side folding
# ----------------------------------------------------------------------------

def fold_params(inp):
    f64 = np.float64
    f = np.asarray(inp["oct_struct"], f64)  # [8,8,8] f[j,k,i]
    W_cat = np.concatenate(
        [np.asarray(inp[k], f64) for k in ("W_sigma", "W_lam", "W_alp")], axis=1
    )  # [1024,48]
    W_all = np.asarray(inp["W_in"], f64) @ W_cat
    b_all = np.asarray(inp["b_in"], f64) @ W_cat + np.concatenate(
        [np.asarray(inp[k], f64) for k in ("b_sigma", "b_lam", "b_alp")]
    )

    # cmul structure tensor G[i,j,k]: cmul(u,v)_i = sum_jk G[i,j,k] u_j v_k
    G = np.zeros((16, 16, 16), f64)
    ft = np.transpose(f, (2, 0, 1))  # ft[i,j,k] = f[j,k,i]
    G[:8, :8, :8] = ft
    G[:8, 8:, 8:] = -ft
    G[8:, :8, 8:] = ft
    G[8:, 8:, :8] = ft
    G2 = G.transpose(1, 2, 0).reshape(256, 16)  # [jk, i]

    JE = np.asarray(inp["J_expand"], f64)
    A = (JE - np.transpose(JE, (0, 2, 1))).reshape(16, 256)

    Gamma = np.einsum("ab,bcd->cd", np.asarray(inp["tetrad"], f64),
                      np.asarray(inp["gammas"], f64))
    sp = np.einsum("gdk,gd->k", np.asarray(inp["Pi_spinor"], f64), Gamma)

    PiS = np.asarray(inp["Pi_source"], f64).reshape(256, 16)
    PiT = np.asarray(inp["Pi_target"], f64).reshape(256, 16)
    C = (A @ PiS) @ PiT.T * np.tile(sp, 16)[None, :]

    kw = np.asarray(inp["kweights"], f64)
    alpha = kw[0]
    W_out = np.asarray(inp["W_out"], f64)
    P1 = alpha * (A @ W_out)
    P2 = (1.0 - alpha) * (C @ W_out)
    b_out = np.asarray(inp["b_out"], f64)

    # wcat [1024, 80] in pall layout: ps@0-15, sumx-ones@16, pl@32-47, pa@64-79
    # (zero pad cols elsewhere so every pall psum row is written -> 1-op evac)
    wcat = np.zeros((DM, 80), f64)
    wcat[:, 0:16] = W_all[:, 0:16]
    wcat[:, 16] = 1.0
    wcat[:, 32:48] = W_all[:, 16:32]
    wcat[:, 64:80] = W_all[:, 32:48]

    # per-partition bias for the pall evac [80,1]
    ball = np.zeros((80, 1), f64)
    ball[0:16, 0] = b_all[0:16]
    ball[32:48, 0] = b_all[16:32]
    ball[64:80, 0] = b_all[32:48]

    # pcat [66, 1024]: 0-15 P1 (Jv), 32-47 P2 (Jc), 65 b_out
    # gap rows 16-31/48-64 are zero -> JJc gap rows only need to be non-NaN
    pcat = np.zeros((66, DM), f64)
    pcat[0:16] = P1
    pcat[32:48] = P2
    pcat[65] = b_out

    # svec [48,1]: row-sums of P1 at 0-15, of P2 at 32-47 (for sum_d out)
    svec = np.zeros((48, 1), f64)
    svec[0:16, 0] = P1.sum(axis=1)
    svec[32:48, 0] = P2.sum(axis=1)
    sumb = float(b_out.sum())

    # sel17 [17,1]: selects the sumx row (16) of pall
    sel17 = np.zeros((17, 1), f64)
    sel17[16, 0] = 1.0

    # expansion matrices, replicated at partition bases 0/32/64
    rrep = np.zeros((16, 256), f64)
    rtile = np.zeros((16, 256), f64)
    for j in range(16):
        for k in range(16):
            rrep[j, j * 16 + k] = 1.0
            rtile[k, j * 16 + k] = 1.0
    rrep3 = np.zeros((128, 256), f64)
    rtile3 = np.zeros((128, 256), f64)
    for base in (0, 32, 64):
        rrep3[base:base + 16] = rrep
        rtile3[base:base + 16] = rtile

    # fused contraction+expansion: rep(G2^T w)_h = sum_c mrep[h,c]^T w[:,c,:]
    #   mrep[h,c][p,po] = G2[c*128+p, (h*128+po)>>4]
    #   mtile[c][p,po]  = G2[c*128+p, po & 15]   (h-independent)
    mrep = np.zeros((128, 4, 128), f64)   # [p, h*2+c, po]
    mtile = np.zeros((128, 2, 128), f64)  # [p, c, po]
    rep_idx = np.array([po >> 4 for po in range(128)])
    til_idx = np.array([po & 15 for po in range(128)])
    for h in range(2):
        for c in range(2):
            mrep[:, h * 2 + c, :] = G2[c * 128:(c + 1) * 128][:, rep_idx + h * 8]
    for c in range(2):
        mtile[:, c, :] = G2[c * 128:(c + 1) * 128][:, til_idx]

    # G2 chunks: [128, 4, 16] = [G2a, G2b, -G2a, -G2b]
    g2c = np.zeros((128, 4, 16), f64)
    g2c[:, 0] = G2[:128]
    g2c[:, 1] = G2[128:]
    g2c[:, 2] = -G2[:128]
    g2c[:, 3] = -G2[128:]

    # conv Toeplitz [192,128]: out[tl] = sum_sl afull[sl, tl] * Jv[t0-64+sl]
    afull = np.zeros((192, 128), f64)
    for sl in range(192):
        for tl in range(128):
            tap = tl + 64 - sl
            if 0 <= tap < KSIZE:
                afull[sl, tl] = kw[tap]
    a1p = np.zeros((128, 128), f64)
    a1p[64:128] = afull[0:64]  # stored at partition base 64

    ln_g = np.asarray(inp["ln_g"], f64)
    ln_b = np.asarray(inp["ln_b"], f64)

    return dict(
        wcat=wcat.astype(BF),
        ball=ball.astype(F32),
        rrep3=rrep3.astype(BF),
        rtile3=rtile3.astype(BF),
        mrep=mrep.astype(BF),
        mtile=mtile.astype(BF),
        g2c=g2c.astype(BF),
        a1p=a1p.astype(BF),
        a2=afull[64:].astype(BF),
        pcat=pcat.astype(BF),
        svec=svec.astype(BF),
        sel17=sel17.astype(BF),
        sumb=sumb,
        ident=np.eye(128).astype(BF),
        ln_g=ln_g.astype(F32),
        ln_b=ln_b.astype(F32),
        g_trivial=bool(np.all(ln_g == 1.0)),
        b_trivial=bool(np.all(ln_b == 0.0)),
    )


# ----------------------------------------------------------------------------
# Device kernel
# ----------------------------------------------------------------------------

def build_kernel(nc, T, sumb, g_trivial, b_trivial, reps=1):
    dt = mybir.dt
    P = 128
    TC = 512                 # token chunk
    TPC = TC // P            # token tiles per chunk (4)
    NCH = T // TC            # chunks
    KT = T // P              # token tiles total

    x_d = nc.declare_dram_parameter("x", [T, DM], dt.float32, isOutput=False)
    y_d = nc.declare_dram_parameter("y", [T, DM], dt.float32, isOutput=True)
    wcat_d = nc.declare_dram_parameter("wcat", [DM, 80], dt.bfloat16, isOutput=False)
    ball_d = nc.declare_dram_parameter("ball", [80, 1], dt.float32, isOutput=False)
    rrep_d = nc.declare_dram_parameter("rrep3", [128, 256], dt.bfloat16, isOutput=False)
    rtile_d = nc.declare_dram_parameter("rtile3", [128, 256], dt.bfloat16, isOutput=False)
    mrep_d = nc.declare_dram_parameter("mrep", [128, 4, 128], dt.bfloat16, isOutput=False)
    mtile_d = nc.declare_dram_parameter("mtile", [128, 2, 128], dt.bfloat16, isOutput=False)
    g2c_d = nc.declare_dram_parameter("g2c", [128, 4, 16], dt.bfloat16, isOutput=False)
    a1p_d = nc.declare_dram_parameter("a1p", [128, 128], dt.bfloat16, isOutput=False)
    a2_d = nc.declare_dram_parameter("a2", [128, 128], dt.bfloat16, isOutput=False)
    pcat_d = nc.declare_dram_parameter("pcat", [66, DM], dt.bfloat16, isOutput=False)
    svec_d = nc.declare_dram_parameter("svec", [48, 1], dt.bfloat16, isOutput=False)
    sel17_d = nc.declare_dram_parameter("sel17", [17, 1], dt.bfloat16, isOutput=False)
    ident_d = nc.declare_dram_parameter("ident", [128, 128], dt.bfloat16, isOutput=False)
    lng_d = nc.declare_dram_parameter("lng", [DM], dt.float32, isOutput=False)
    lnb_d = nc.declare_dram_parameter("lnb", [DM], dt.float32, isOutput=False)

    with tile.TileContext(nc) as tc, ExitStack() as ctx:
        consts = ctx.enter_context(tc.tile_pool(name="consts", bufs=1))
        persist = ctx.enter_context(tc.tile_pool(name="persist", bufs=1))
        xin = ctx.enter_context(tc.tile_pool(name="xin", bufs=3))
        xbp = ctx.enter_context(tc.tile_pool(name="xbp", bufs=3))
        xtp = ctx.enter_context(tc.tile_pool(name="xtp", bufs=2))
        mid = ctx.enter_context(tc.tile_pool(name="mid", bufs=2))
        jjp = ctx.enter_context(tc.tile_pool(name="jjp", bufs=2))
        ycp = ctx.enter_context(tc.tile_pool(name="ycp", bufs=2))
        stat = ctx.enter_context(tc.tile_pool(name="stat", bufs=2))
        psA = ctx.enter_context(tc.tile_pool(name="psA", bufs=3, space="PSUM"))
        psS = ctx.enter_context(tc.tile_pool(name="psS", bufs=2, space="PSUM"))
        psO = ctx.enter_context(tc.tile_pool(name="psO", bufs=3, space="PSUM"))

        # ---- constants into SBUF ----
        wcat_sb = consts.tile([P, 8, 80], dt.bfloat16)
        nc.sync.dma_start(wcat_sb[:], wcat_d.rearrange("(a p) m -> p a m", p=P))
        ball_sb = consts.tile([80, 1], dt.float32)
        nc.sync.dma_start(ball_sb[:], ball_d[:])
        rrep_sb = consts.tile([128, 256], dt.bfloat16)
        nc.sync.dma_start(rrep_sb[:], rrep_d[:])
        rtile_sb = consts.tile([128, 256], dt.bfloat16)
        nc.sync.dma_start(rtile_sb[:], rtile_d[:])
        mrep_sb = consts.tile([128, 4, 128], dt.bfloat16)
        nc.sync.dma_start(mrep_sb[:], mrep_d[:])
        mtile_sb = consts.tile([128, 2, 128], dt.bfloat16)
        nc.sync.dma_start(mtile_sb[:], mtile_d[:])
        g2_sb = consts.tile([128, 4, 16], dt.bfloat16)
        nc.sync.dma_start(g2_sb[:], g2c_d[:])
        a1p_sb = consts.tile([128, 128], dt.bfloat16)
        nc.sync.dma_start(a1p_sb[:], a1p_d[:])
        a2_sb = consts.tile([128, 128], dt.bfloat16)
        nc.sync.dma_start(a2_sb[:], a2_d[:])
        pcat_sb = consts.tile([66, DM], dt.bfloat16)
        nc.sync.dma_start(pcat_sb[:], pcat_d[:])
        svec_sb = consts.tile([48, 1], dt.bfloat16)
        nc.sync.dma_start(svec_sb[:], svec_d[:])
        sel17_sb = consts.tile([17, 1], dt.bfloat16)
        nc.sync.dma_start(sel17_sb[:], sel17_d[:])
        ident_sb = consts.tile([128, 128], dt.bfloat16)
        nc.sync.dma_start(ident_sb[:], ident_d[:])

        gb_sb = None
        if not (g_trivial and b_trivial):
            gb_sb = consts.tile([P, 2, DM], dt.float32)
            nc.sync.dma_start(gb_sb[:, 0, :], lng_d[None, :].to_broadcast((P, DM)))
            nc.sync.dma_start(gb_sb[:, 1, :], lnb_d[None, :].to_broadcast((P, DM)))

        # ---- helpers ----
        def load_x(c):
            x32 = xin.tile([P, TPC, DM], dt.float32, tag="x32")
            nc.sync.dma_start(
                x32[:],
                x_d[c * TC:(c + 1) * TC, :].rearrange("(j p) d -> p j d", p=P),
            )
            xb = xbp.tile([P, TPC, DM], dt.bfloat16, tag="xb")
            for j in range(TPC):
                nc.gpsimd.tensor_copy(xb[:, j, :], x32[:, j, :])
            return x32, xb

        def emit_transp_pe(xb, j):
            pxT = psA.tile([P, 8, 128], dt.bfloat16, tag="psA")
            for a in range(8):
                nc.tensor.transpose(
                    pxT[:, a, :], xb[:, j, bass.ts(a, 128)], ident_sb[:]
                )
            return pxT

        def emit_transp_evac(pxT, xT, j):
            # alternate engines so the Act queue in the LN tail stays short
            if j % 2 == 0:
                nc.scalar.copy(xT[:, :, bass.ts(j, 128)], pxT[:])
            else:
                nc.vector.tensor_copy(xT[:, :, bass.ts(j, 128)], pxT[:])

        def emit_proj(xT):
            pps = psS.tile([80, TC], dt.float32, tag="psS")
            for a in range(8):
                nc.tensor.matmul(
                    pps[0:80, :], wcat_sb[:, a, 0:80], xT[:, a, :],
                    start=(a == 0), stop=(a == 7),
                )
            pall = mid.tile([80, TC], dt.bfloat16, tag="pall")
            nc.vector.tensor_scalar(
                pall[:], pps[:], ball_sb[:], None, mybir.AluOpType.add
            )
            return pall

        # ---- persistent + prologue ----
        rep_cm = tc.For_i(0, reps, 1) if reps > 1 else None
        if rep_cm is not None:
            rep_cm.__enter__()
        JvT = persist.tile([P, KT, 16], dt.bfloat16)  # token-major Jv

        _, xb_cur = load_x(0)
        xb_nxt = load_x(1)[1] if NCH > 1 else None
        xT0 = xtp.tile([P, 8, TC], dt.bfloat16, tag="xT")
        for j in range(TPC):
            pxT = emit_transp_pe(xb_cur, j)
            emit_transp_evac(pxT, xT0, j)
        pall_cur = emit_proj(xT0)

        for c in range(NCH):
            t0 = c * TC
            # ---- JJ gap rows on Pool, then prefetch chunk c+1 ----
            # gpsimd needs 32-aligned partition bases; row 64 becomes 1.0
            # alongside row 65, which is harmless because pcat row 64 is zero
            JJc = jjp.tile([66, TC], dt.bfloat16, tag="JJc")
            nc.gpsimd.memset(JJc[0:64, :], 0.0)
            nc.gpsimd.memset(JJc[64:66, :], 1.0)
            # prefetch x two chunks ahead so the Pool convert is always
            # finished before chunk c+1's transposes need it
            xb_n2 = load_x(c + 2)[1] if c + 2 < NCH else None

            pall = pall_cur
            xb = xb_cur
            ps_ap = pall[0:16, :]
            pl_ap = pall[32:48, :]
            pa_ap = pall[64:80, :]

            # ---- associator ----
            # DVE may read only one non-scalar PSUM input per op, so each
            # tile-expansion factor is evacuated to SBUF before the multiply.
            # w1 = rep(ps) * tile(pl)
            w1 = mid.tile([P, 2, TC], dt.bfloat16, tag="w1")
            vt_pl = mid.tile([P, 2, TC], dt.bfloat16, tag="vtpl")
            for h in range(2):
                ptile = psA.tile([P, TC], dt.float32, tag="psA")
                nc.tensor.matmul(
                    ptile[:], rtile_sb[32:48, bass.ts(h, 128)], pl_ap)
                nc.vector.tensor_copy(vt_pl[:, h, :], ptile[:])
                prep = psA.tile([P, TC], dt.float32, tag="psA")
                nc.tensor.matmul(
                    prep[:], rrep_sb[0:16, bass.ts(h, 128)], ps_ap)
                nc.vector.tensor_mul(w1[:, h, :], prep[:], vt_pl[:, h, :])

            # w2 = rep(pl) * tile(pa); tile(pa) also evacuated for w3 reuse
            w2 = mid.tile([P, 2, TC], dt.bfloat16, tag="w2")
            vt_pa = mid.tile([P, 2, TC], dt.bfloat16, tag="vtpa")
            for h in range(2):
                ptile = psA.tile([P, TC], dt.float32, tag="psA")
                nc.tensor.matmul(
                    ptile[:], rtile_sb[64:80, bass.ts(h, 128)], pa_ap)
                nc.scalar.copy(vt_pa[:, h, :], ptile[:])
                prep = psA.tile([P, TC], dt.float32, tag="psA")
                nc.tensor.matmul(
                    prep[:], rrep_sb[32:48, bass.ts(h, 128)], pl_ap)
                nc.vector.tensor_mul(w2[:, h, :], prep[:], vt_pa[:, h, :])

            # w3 = rep(U) * tile(pa), U = G2^T w1 fused into mrep matmuls
            w3 = mid.tile([P, 2, TC], dt.bfloat16, tag="w3")
            for h in range(2):
                prep = psA.tile([P, TC], dt.float32, tag="psA")
                nc.tensor.matmul(prep[:], mrep_sb[:, 2 * h + 0, :], w1[:, 0, :],
                                 start=True, stop=False)
                nc.tensor.matmul(prep[:], mrep_sb[:, 2 * h + 1, :], w1[:, 1, :],
                                 start=False, stop=True)
                nc.vector.tensor_mul(w3[:, h, :], prep[:], vt_pa[:, h, :])

            # w4 = rep(ps) * tile(Y), Y = G2^T w2 fused into mtile matmuls
            w4 = mid.tile([P, 2, TC], dt.bfloat16, tag="w4")
            vt_y = mid.tile([P, 2, TC], dt.bfloat16, tag="vty")
            for h in range(2):
                ptile = psA.tile([P, TC], dt.float32, tag="psA")
                nc.tensor.matmul(ptile[:], mtile_sb[:, 0, :], w2[:, 0, :],
                                 start=True, stop=False)
                nc.tensor.matmul(ptile[:], mtile_sb[:, 1, :], w2[:, 1, :],
                                 start=False, stop=True)
                nc.vector.tensor_copy(vt_y[:, h, :], ptile[:])
                prep = psA.tile([P, TC], dt.float32, tag="psA")
                nc.tensor.matmul(
                    prep[:], rrep_sb[0:16, bass.ts(h, 128)], ps_ap)
                nc.vector.tensor_mul(w4[:, h, :], prep[:], vt_y[:, h, :])

            # ---- Jv token-major directly (w stationary, G2 chunks moving) ----
            # emitted first so the conv below starts as early as possible
            pjvT = psS.tile([P, TPC, 16], dt.float32, tag="psS")
            for j in range(TPC):
                for i, (gi, w, h) in enumerate(
                        [(0, w3, 0), (1, w3, 1), (2, w4, 0), (3, w4, 1)]):
                    nc.tensor.matmul(
                        pjvT[:, j, :], w[:, h, bass.ts(j, 128)],
                        g2_sb[:, gi, :],
                        start=(i == 0), stop=(i == 3),
                    )
            nc.scalar.copy(JvT[:, c * TPC:(c + 1) * TPC, :], pjvT[:])

            # ---- Jv feature-major -> JJc rows 0-15 ----
            pJv = psS.tile([16, TC], dt.float32, tag="psS")
            for i, (gi, w, h) in enumerate(
                    [(0, w3, 0), (1, w3, 1), (2, w4, 0), (3, w4, 1)]):
                nc.tensor.matmul(
                    pJv[:], g2_sb[:, gi, :], w[:, h, :],
                    start=(i == 0), stop=(i == 3),
                )
            nc.scalar.copy(JJc[0:16, :], pJv[:])

            # ---- causal conv (Toeplitz matmuls) -> Jc at rows 32-47 ----
            pJc = psS.tile([48, TC], dt.float32, tag="psS")
            for j in range(TPC):
                g = c * TPC + j
                osl = pJc[32:48, bass.ts(j, 128)]
                if g > 0:
                    nc.tensor.matmul(
                        osl, JvT[64:128, g - 1, :], a1p_sb[64:128, :],
                        start=True, stop=False, tile_position=(64, 32),
                    )
                    nc.tensor.matmul(
                        osl, JvT[:, g, :], a2_sb[:],
                        start=False, stop=True, tile_position=(0, 32),
                    )
                else:
                    nc.tensor.matmul(
                        osl, JvT[:, g, :], a2_sb[:],
                        start=True, stop=True, tile_position=(0, 32),
                    )
            nc.scalar.copy(JJc[32:48, :], pJc[32:48, :])

            # ---- mean, token-major: -mu via JJc/pall stationaries ----
            # (runs in parallel with the final matmuls, off their path)
            pmuT = psS.tile([P, TPC], dt.float32, tag="psS")
            for j in range(TPC):
                nc.tensor.matmul(
                    pmuT[:, j:j + 1], JJc[0:48, bass.ts(j, 128)], svec_sb[:],
                    start=True, stop=False,
                )
                nc.tensor.matmul(
                    pmuT[:, j:j + 1], pall[0:17, bass.ts(j, 128)], sel17_sb[:],
                    start=False, stop=True,
                )
            negmu = stat.tile([P, TPC], dt.float32, tag="negmu")
            nc.scalar.activation(
                negmu[:], pmuT[:], mybir.ActivationFunctionType.Copy,
                bias=-sumb / DM, scale=-1.0 / DM,
            )
            m2e = stat.tile([P, TPC], dt.float32, tag="m2e")
            nc.vector.tensor_mul(m2e[:], negmu[:], negmu[:])
            nc.vector.tensor_scalar(
                m2e[:], m2e[:], -1.0, float(EPS),
                mybir.AluOpType.mult, mybir.AluOpType.add,
            )

            # ---- output phase, interleaved with chunk c+1 front-end ----
            xT_nxt = None
            if xb_nxt is not None:
                xT_nxt = xtp.tile([P, 8, TC], dt.bfloat16, tag="xT")
            yc = ycp.tile([P, TPC, DM], dt.float32)
            ssqA = stat.tile([P, TPC], dt.float32, tag="ssqA")
            ssqT = stat.tile([P, TPC], dt.float32, tag="ssqT")
            sg4 = stat.tile([P, TPC], dt.float32, tag="sg4")
            rs4 = stat.tile([P, TPC], dt.float32, tag="rs4")
            mrsn = stat.tile([P, TPC], dt.float32, tag="mrsn")
            for j in range(TPC):
                pouts = []
                pxT = None
                if xT_nxt is not None:
                    pxT = psA.tile([P, 8, 128], dt.bfloat16, tag="psA", name="pxT")
                for nh in range(2):
                    pout = psO.tile([P, 512], dt.float32, tag="psO")
                    nc.tensor.matmul(
                        pout[:], ident_sb[:], xb[:, j, bass.ts(nh, 512)],
                        start=True, stop=False,
                    )
                    nc.tensor.matmul(
                        pout[:], JJc[0:66, bass.ts(j, 128)],
                        pcat_sb[:, bass.ts(nh, 512)],
                        start=False, stop=True,
                    )
                    pouts.append(pout)
                    if pxT is not None:
                        for a in range(nh * 4, nh * 4 + 4):
                            nc.tensor.transpose(
                                pxT[:, a, :], xb_nxt[:, j, bass.ts(a, 128)],
                                ident_sb[:],
                            )
                # DVE cannot read the same PSUM tile twice, so the squares
                # run on Act (single-input Square with accumulator)
                sqs = stat.tile([P, 512], dt.bfloat16, tag="sqs")
                nc.scalar.activation(
                    sqs[:], pouts[0][:], mybir.ActivationFunctionType.Square,
                    accum_out=ssqA[:, j:j + 1],
                )
                sqs2 = stat.tile([P, 512], dt.bfloat16, tag="sqs")
                nc.scalar.activation(
                    sqs2[:], pouts[1][:], mybir.ActivationFunctionType.Square,
                    accum_out=ssqT[:, j:j + 1],
                )
                nc.vector.tensor_add(
                    ssqT[:, j:j + 1], ssqT[:, j:j + 1], ssqA[:, j:j + 1])
                nc.scalar.activation(
                    sg4[:, j:j + 1], ssqT[:, j:j + 1],
                    mybir.ActivationFunctionType.Sqrt,
                    bias=m2e[:, j:j + 1], scale=1.0 / DM,
                )
                nc.vector.reciprocal(rs4[:, j:j + 1], sg4[:, j:j + 1])
                nc.vector.tensor_mul(
                    mrsn[:, j:j + 1], negmu[:, j:j + 1], rs4[:, j:j + 1])
                # normalize doubles as the PSUM->SBUF evacuation
                nc.scalar.activation(
                    yc[:, j, bass.ts(0, 512)], pouts[0][:],
                    mybir.ActivationFunctionType.Identity,
                    scale=rs4[:, j:j + 1], bias=mrsn[:, j:j + 1],
                )
                nc.vector.tensor_scalar(
                    yc[:, j, bass.ts(1, 512)], pouts[1][:],
                    rs4[:, j:j + 1], mrsn[:, j:j + 1],
                    mybir.AluOpType.mult, mybir.AluOpType.add,
                )
                if gb_sb is not None:
                    nc.vector.tensor_mul(yc[:, j, :], yc[:, j, :], gb_sb[:, 0, :])
                    nc.vector.tensor_add(yc[:, j, :], yc[:, j, :], gb_sb[:, 1, :])
                if pxT is not None:
                    emit_transp_evac(pxT, xT_nxt, j)
                # y stores go out on the gpsimd SWDGE queue so they never
                # queue ahead of the next chunk's x load on the SP queue
                nc.gpsimd.dma_start(
                    y_d[t0 + j * P:t0 + (j + 1) * P, :], yc[:, j, :]
                )

            if xT_nxt is not None:
                pall_cur = emit_proj(xT_nxt)
                xb_cur = xb_nxt
            xb_nxt = xb_n2

        if rep_cm is not None:
            rep_cm.__exit__(None, None, None)

    return nc


# ----------------------------------------------------------------------------
# Entry point
# ----------------------------------------------------------------------------

def _const_map(fp):
    return {
        "wcat": fp["wcat"], "ball": fp["ball"], "rrep3": fp["rrep3"],
        "rtile3": fp["rtile3"], "mrep": fp["mrep"], "mtile": fp["mtile"],
        "g2c": fp["g2c"], "a1p": fp["a1p"],
        "a2": fp["a2"], "pcat": fp["pcat"], "svec": fp["svec"],
        "sel17": fp["sel17"], "ident": fp["ident"],
        "lng": fp["ln_g"], "lnb": fp["ln_b"],
    }


def _run(inputs, trace=False):
    x = inputs["x"]
    assert x.shape == (B, N, DM), x.shape
    fp = fold_params(inputs)

    nc = bacc.Bacc("TRN2", target_bir_lowering=False)
    build_kernel(nc, N, fp["sumb"], fp["g_trivial"], fp["b_trivial"])
    nc.finalize()

    cm = _const_map(fp)
    in_maps = [
        {"x": np.ascontiguousarray(x[i], dtype=F32), **cm} for i in range(NCORES)
    ]
    return run_bass_kernel_spmd(nc, in_maps, list(range(NCORES)), trace=trace)


def kernel(**inputs):
    inputs = {k: np.asarray(v) for k, v in inputs.items()}
    res = _run(inputs)
    y = np.stack([res.results[i]["y"] for i in range(NCORES)], axis=0)
    return y.astype(np.float32)


def timed_run(inputs):
    """NTFF profiling is unavailable under axon in this container; timing is
    done by test.py via repeated execution of an in-kernel repeat loop."""
    return None


if __name__ == "__main__":
    import reference

    inp = reference.setup_inputs()
    out = kernel(**{k: np.asarray(v) for k, v in inp.items()})
    print("kernel output", out.shape, out.dtype)

